# revision 41
# baseline (speedup 1.0000x reference)
"""NestedAttention Trainium2 kernel (v3: row-tiled mm1, fp8 DoubleRow mm2,
exp split across Scalar+Vector engines per n-chunk).

Reference computation (per batch b):
  q_i = wq[i] @ x ; k_j = wk[j] @ x ; v_j = wv[j] @ x        (1x1 convs, r=64)
  for i: acc_i = sum_j softmax_m(q_i^T k_j / sqrt(r)) applied to v_j
  out = wo @ concat_i(acc_i) ; y = x * sigmoid(out)

Sharding: 8 cores = batch(4) x query-column-halves(2). Each core holds full
k/v (m = 2304 keys) and a 1152-wide slice of query columns n; no cross-core
communication (softmax is over m, fully on-core).

Per-core dataflow:
  mm1  T_ij[m, n] = k_j^T q_i  -- K=64 contraction, so the PE runs TWO
       concurrent 64x128 row-tiles (tile_position (0,0)/(64,0)); q/k are
       duplicated into partitions 64:128 (SBUF->SBUF DMA) to feed tile 1.
       Each (m-pair, chunk) lands in one 2-bank PSUM ring slot [128, 2, cw].
  exp  E = exp(0.125*T - 1.5), engine assigned PER N-CHUNK so ScalarE and
       VectorE drain ring slots concurrently:
         chunk0 (n 0:512)    -> ScalarE exact exp -> fp8e4
         chunk1 (n 512:1024) -> VectorE Schraudolph (int8 bitcast fp8e5),
                                except CH1_ACT m-pairs on ScalarE -> fp8e4
         chunk2 (n 1024:1152)-> ScalarE -> fp8e4
       The -1.5 bias cancels in softmax and keeps E < 240 (TRN e4m3 max).
  mm2  [vT_j | ones]^T @ E -- fp8 DoubleRow over m-slab pairs (FD=512
       chunks; the 128-wide chunk uses plain fp8 matmuls).  Rows 0:64 = v@E,
       rows 64:128 = Z (softmax denominator).
  norm acc_i += (v@E) * (1/Z): Z copy + reciprocal + multiply on VectorE,
       the j>0 accumulate add on GpSimd (Pool).
  out  wo^T @ acc (bf16), sigmoid (ScalarE), x*sig (VectorE), DMA out.
"""

import os
import numpy as np

B, C, H, W = 4, 256, 48, 48
N = H * W            # 2304 keys (m) per image
NSLICE = N // 2      # 1152 query columns (n) per core
R = 64               # reduced channels
P = 128
MT = N // P          # 18 m-tiles
MP = MT // 2         # 9 m-tile pairs (row-tiled mm1 does 2 at once)
KT = C // P          # 2 contraction tiles over channels
CHUNKS = [(0, 512), (512, 512), (1024, 128)]
N_CORES = 8

# chunk1 m-pairs whose exp runs on ScalarE instead of VectorE (load balance)
CH1_ACT = tuple(
    int(v) for v in os.environ.get("NESTED_CH1_ACT", "8").split(",") if v != ""
)
# chunk2 m-pairs whose exp runs on VectorE (default: none, all ScalarE)
CH2_DVE = tuple(
    int(v) for v in os.environ.get("NESTED_CH2_DVE", "").split(",") if v != ""
)

# exp(0.125*T + EBIAS); EBIAS cancels in softmax, gives fp8e4 overflow margin
EBIAS = -1.5
LOG2E = 1.4426950408889634
SCHRAUD_A = 4 * 0.125 * LOG2E
# 4*(15 + EBIAS*log2e) - 0.172 (sawtooth centering) + 0.25 (trunc/round split)
SCHRAUD_B = 4 * (15.0 + EBIAS * LOG2E) - 0.172 + 0.25

COPY1 = os.environ.get("NESTED_COPY1", "1") == "1"     # single 64-row Z copy
MIXED_DR = os.environ.get("NESTED_MIXED_DR", "0") == "1"  # e4 lhsT + e5 rhs

_CACHE = {}
LAST_RESULTS = None


def _build_program():
    from contextlib import ExitStack

    import concourse.bass as bass
    import concourse.tile as tile
    from concourse import bacc, mybir

    f32 = mybir.dt.float32
    bf16 = mybir.dt.bfloat16
    f8e4 = mybir.dt.float8e4
    f8e5 = mybir.dt.float8e5
    i8 = mybir.dt.int8
    Exp = mybir.ActivationFunctionType.Exp
    Sigmoid = mybir.ActivationFunctionType.Sigmoid
    Copy = mybir.ActivationFunctionType.Copy
    mult = mybir.AluOpType.mult
    add = mybir.AluOpType.add
    DR = mybir.MatmulPerfMode.DoubleRow

    nc = bacc.Bacc("TRN2", target_bir_lowering=False, debug=False)
    # fp8 inputs: the q/k/v projections contract K=256 = 2 slabs of 128, a
    # genuine fp8 DoubleRow case (one matmul per chunk instead of two)
    xb_d = nc.declare_dram_parameter("xb", [KT, P, N], f8e4, isOutput=False)
    xnb_d = nc.declare_dram_parameter("xnb", [KT, P, NSLICE], f8e4, isOutput=False)
    xn_d = nc.declare_dram_parameter("xn", [KT, P, NSLICE], f32, isOutput=False)
    # wq/wk transposed + zero-padded to 128 output cols so proj matmuls keep
    # the 128x128 PE mode (out partitions = 128, rows 64:128 zero)
    wqT_d = nc.declare_dram_parameter("wqT", [KT, P, 3, P], f8e4, isOutput=False)
    wkT_d = nc.declare_dram_parameter("wkT", [KT, P, 3, P], f8e4, isOutput=False)
    wvT_d = nc.declare_dram_parameter("wvT", [KT, P, 3, R], f8e4, isOutput=False)
    woT_d = nc.declare_dram_parameter("woT", [3, R, C], bf16, isOutput=False)
    y_d = nc.declare_dram_parameter("y", [KT, P, NSLICE], f32, isOutput=True)

    with tile.TileContext(nc) as tc, ExitStack() as ctx:
        consts = ctx.enter_context(tc.tile_pool(name="consts", bufs=1))
        ring = ctx.enter_context(tc.tile_pool(name="ring", bufs=3, space="PSUM"))
        mm2_ps = ctx.enter_context(tc.tile_pool(name="mm2_ps", bufs=2, space="PSUM"))
        e_pool = ctx.enter_context(tc.tile_pool(name="e_pool", bufs=3))
        small = ctx.enter_context(tc.tile_pool(name="small", bufs=2))

        # PE warm-up: dummy matmuls ramp the HAM clock gate to 2.4 GHz while
        # the framework preamble and input DMAs are still running.
        warm = consts.tile([P, 512], bf16, name="warm")
        nc.gpsimd.memset(warm[:], 0.0)
        for w in range(12):
            wsl = ring.tile([P, 2, 512], f32, tag="ring", name=f"wu{w}")
            nc.tensor.matmul(
                wsl[:, 0, :], warm[:, 0:128], warm[:], start=True, stop=True
            )

        # ---- persistent SBUF state ----
        wqT_sb = consts.tile([P, KT, 3, P], f8e4)
        nc.sync.dma_start(wqT_sb[:], wqT_d.rearrange("t p i r -> p t i r"))
        xnb_sb = consts.tile([P, KT, NSLICE], f8e4)
        nc.sync.dma_start(xnb_sb[:], xnb_d.rearrange("t p m -> p t m"))
        wkT_sb = consts.tile([P, KT, 3, P], f8e4)
        nc.sync.dma_start(wkT_sb[:], wkT_d.rearrange("t p i r -> p t i r"))
        x_sb = consts.tile([P, KT, N], f8e4)
        for q4 in range(2):
            n0, n1 = q4 * NSLICE, (q4 + 1) * NSLICE
            nc.sync.dma_start(
                x_sb[:, :, n0:n1], xb_d[:, :, n0:n1].rearrange("t p m -> p t m")
            )
        wvT_sb = consts.tile([P, KT, 3, R], f8e4)
        nc.sync.dma_start(wvT_sb[:], wvT_d.rearrange("t p i r -> p t i r"))
        xn_sb = consts.tile([P, KT, NSLICE], f32)
        for q4 in range(2):
            n0, n1 = q4 * 576, (q4 + 1) * 576
            nc.sync.dma_start(
                xn_sb[:, :, n0:n1], xn_d[:, :, n0:n1].rearrange("t p m -> p t m")
            )

        woT_sb = []
        for i in range(3):
            w = consts.tile([P, C], bf16, tag=f"woT{i}", name=f"woT{i}")
            nc.gpsimd.memset(w[R:P, :], 0.0)
            nc.sync.dma_start(w[0:R, :], woT_d[i])
            woT_sb.append(w)

        # q/k: rows 0:64 = values, rows 64:128 = duplicate (feeds row-tile 1)
        q_sb = consts.tile([P, 3, NSLICE], bf16)
        k_sb = consts.tile([P, 3, N], bf16)

        # vT buffers per m-pair: [2 slabs, [vT_0|ones|vT_1|ones|vT_2|ones]]
        # e4m3 copy feeds chunk0/2 matmuls, e5m2 copy feeds chunk1.
        vT4 = consts.tile([P, MP, 2, 384], f8e4)
        for j in range(3):
            nc.gpsimd.memset(vT4[:, :, :, 128 * j + 64 : 128 * j + 128], 1.0)
        if not MIXED_DR:
            vT5 = consts.tile([P, MP, 2, 384], f8e5)
            for j in range(3):
                nc.gpsimd.memset(vT5[:, :, :, 128 * j + 64 : 128 * j + 128], 1.0)

        ebias_ap = consts.tile([P, 1], f32, name="ebias")
        nc.gpsimd.memset(ebias_ap[:], EBIAS)

        # acc_i accumulated in bf16; rows 64:128 zero (K=128 pad for final mm)
        acc = []
        for i in range(3):
            a = consts.tile([P, NSLICE], bf16, tag=f"acc{i}", name=f"acc{i}")
            nc.gpsimd.memset(a[R:P, :], 0.0)
            acc.append(a)

        # ---- projections (use the mm2 PSUM pool; never touch the mm1 ring) ----
        def _proj_1152(wT_sb, wi, src_sb, src_base, dst, name):
            """dst[0:64, 0:1152] (strided AP ok) = w.T @ src columns.
            fp8 DoubleRow over the two K=128 channel slabs."""
            for ci, (c0, cw) in enumerate(CHUNKS):
                pt = mm2_ps.tile([P, 512], f32, tag="mm2", name=f"{name}{ci}")
                nc.tensor.matmul(
                    pt[:, 0:cw],
                    wT_sb[:, :, wi, :],
                    src_sb[:, :, src_base + c0 : src_base + c0 + cw],
                    start=True,
                    stop=True,
                    perf_mode=DR,
                )
                nc.vector.tensor_copy(dst[0:R, c0 : c0 + cw], pt[0:R, 0:cw])

        def emit_q(i):
            _proj_1152(wqT_sb, i, xnb_sb, 0, q_sb[:, i, :], f"qp{i}")
            nc.sync.dma_start(q_sb[R:P, i, :], q_sb[0:R, i, :])

        def emit_k_half(j, half):
            n0 = half * NSLICE
            _proj_1152(wkT_sb, j, x_sb, n0, k_sb[:, j, n0 : n0 + NSLICE], f"kp{j}{half}")
            nc.sync.dma_start(
                k_sb[R:P, j, n0 : n0 + NSLICE], k_sb[0:R, j, n0 : n0 + NSLICE]
            )

        def emit_vT(mt):
            mp, s = mt // 2, mt % 2
            pv = mm2_ps.tile([P, 512], f32, tag="mm2", name=f"pv{mt}")
            nc.tensor.matmul(
                pv[:, 0 : 3 * R],
                x_sb[:, :, mt * P : (mt + 1) * P],
                wvT_sb[:, :, :, :],
                start=True,
                stop=True,
                perf_mode=DR,
            )
            src = pv[:, 0 : 3 * R].rearrange("p (j r) -> p j r", j=3)
            targets = [(vT4, True)] if MIXED_DR else [(vT4, True), (vT5, False)]
            for buf, on_act in targets:
                base = buf[:, mp, s, :]
                dst = bass.AP(
                    tensor=base.tensor,
                    offset=base.offset,
                    ap=[base.ap[0], [128, 3], [1, R]],
                )
                if on_act:
                    nc.scalar.activation(dst, src, Copy)
                else:
                    nc.vector.tensor_copy(dst, src)

        # ---- attention ----
        c2_state = {}

        def emit_mm1_exp_mp(i, j, E4c0, E5c1, E4c2, mp):
            """Row-tiled mm1 (2 concurrent 64x128 tiles) + per-slot exp for
            one m-pair.

            E layouts keep each exp instruction's input AND output fully
            contiguous: E4c0/E5c1 are [P, mp, 1024] = [tileA 512 | tileB 512],
            E4c2 is [P, mp, 256] = [tileA 128 | tileB 128].  The 128-wide
            chunk2 outputs of four m-pairs park in ONE ring slot and share a
            single exp instruction (amortizes ScalarE access latency).
            """
            mA, mB = 2 * mp, 2 * mp + 1
            for ci, (c0, cw) in enumerate(CHUNKS[:2]):
                sl = ring.tile(
                    [P, 2, 512], f32, tag="ring", name=f"t{i}{j}m{mp}c{c0}"
                )
                nc.tensor.matmul(
                    sl[:, 0, 0:cw],
                    k_sb[0:R, j, mA * P : (mA + 1) * P],
                    q_sb[0:R, i, c0 : c0 + cw],
                    start=True,
                    stop=True,
                    tile_position=(0, 0),
                )
                nc.tensor.matmul(
                    sl[:, 1, 0:cw],
                    k_sb[R:P, j, mB * P : (mB + 1) * P],
                    q_sb[R:P, i, c0 : c0 + cw],
                    start=True,
                    stop=True,
                    tile_position=(64, 0),
                )
                on_dve = (ci == 1) and (mp not in CH1_ACT)
                src = sl[:, :, :].rearrange("p s c -> p (s c)")
                if ci == 0:
                    dst4 = E4c0[:, mp, :]
                    dst5 = dst4.bitcast(i8)
                else:
                    dst5 = E5c1[:, mp, :]
                    dst4 = dst5.bitcast(f8e4)
                if on_dve:
                    nc.vector.tensor_scalar(
                        dst5, src, SCHRAUD_A, SCHRAUD_B, mult, add
                    )
                else:
                    nc.scalar.activation(
                        dst4, src, Exp, bias=ebias_ap[:], scale=0.125
                    )
            # chunk2 (n 1024:1152), parked 4 m-pairs per slot
            c0, cw = CHUNKS[2]
            g, gi = mp // 4, mp % 4
            if gi == 0:
                c2_state["sl"] = ring.tile(
                    [P, 2, 512], f32, tag="ring", name=f"t{i}{j}g{g}c2"
                )
            sl = c2_state["sl"]
            nc.tensor.matmul(
                sl[:, 0, 128 * gi : 128 * gi + cw],
                k_sb[0:R, j, mA * P : (mA + 1) * P],
                q_sb[0:R, i, c0 : c0 + cw],
                start=True,
                stop=True,
                tile_position=(0, 0),
            )
            nc.tensor.matmul(
                sl[:, 1, 128 * gi : 128 * gi + cw],
                k_sb[R:P, j, mB * P : (mB + 1) * P],
                q_sb[R:P, i, c0 : c0 + cw],
                start=True,
                stop=True,
                tile_position=(64, 0),
            )
            if mp in (3, 7, 8):
                gsize = gi + 1
                src = sl[:, :, 0 : 128 * gsize].rearrange(
                    "p s (g c) -> p s g c", c=128
                )
                base = E4c2[:, 4 * g, :]
                dst4 = bass.AP(
                    tensor=base.tensor,
                    offset=base.offset,
                    ap=[base.ap[0], [128, 2], [256, gsize], [1, 128]],
                )
                nc.scalar.activation(
                    dst4, src, Exp, bias=ebias_ap[:], scale=0.125
                )

        mm2_state = {}

        def emit_mm2_part(i, j, E4c0, E5c1, E4c2, part, last=False):
            """One third of a chunk's mm2 accumulation chain (interleaved
            between m-pairs so the PE never bursts long enough to drain the
            mm1 ring); the norm runs on the closing part."""
            ci, step = part // 3, part % 3
            c0, cw = CHUNKS[ci]
            if step == 0:
                mm2_state[ci] = mm2_ps.tile(
                    [P, 512], f32, tag="mm2", name=f"pa{i}{j}{c0}"
                )
            pa = mm2_state[ci]
            if ci < 2:
                Ec = E4c0 if ci == 0 else E5c1
                for mp in range(3 * step, 3 * step + 3):
                    dve = (ci == 1) and (mp not in CH1_ACT)
                    slabs = Ec[:, mp, :].rearrange("p (s c) -> p s c", s=2)
                    if dve:
                        lhsT = (vT4 if MIXED_DR else vT5)[
                            :, mp, :, 128 * j : 128 * (j + 1)
                        ]
                        rhs = slabs.bitcast(f8e5)
                    else:
                        lhsT = vT4[:, mp, :, 128 * j : 128 * (j + 1)]
                        rhs = slabs.bitcast(f8e4) if ci == 1 else slabs
                    nc.tensor.matmul(
                        pa[:, 0:cw],
                        lhsT,
                        rhs,
                        start=(mp == 0),
                        stop=(mp == MP - 1),
                        perf_mode=DR,
                    )
            else:
                # FD=128: DoubleRow loses to FWL; plain fp8 per slab
                for mt in range(6 * step, 6 * step + 6):
                    mp, s = mt // 2, mt % 2
                    if mp in CH2_DVE:
                        lhsT = (vT4 if MIXED_DR else vT5)[
                            :, mp, s, 128 * j : 128 * (j + 1)
                        ]
                        rhs = E4c2[:, mp, 128 * s : 128 * (s + 1)].bitcast(f8e5)
                    else:
                        lhsT = vT4[:, mp, s, 128 * j : 128 * (j + 1)]
                        rhs = E4c2[:, mp, 128 * s : 128 * (s + 1)]
                    nc.tensor.matmul(
                        pa[:, 0:cw],
                        lhsT,
                        rhs,
                        start=(mt == 0),
                        stop=(mt == MT - 1),
                    )
            if step < 2:
                return
            rb = small.tile([R, 512], f32, tag="rb", name=f"rb{i}{j}{c0}")
            if COPY1:
                nc.vector.tensor_copy(rb[0:R, 0:cw], pa[R:P, 0:cw])
            else:
                nc.vector.tensor_copy(rb[0:32, 0:cw], pa[64:96, 0:cw])
                nc.vector.tensor_copy(rb[32:64, 0:cw], pa[96:128, 0:cw])
            nc.vector.reciprocal_approx_fast(rb[:, 0:cw], rb[:, 0:cw])
            if j == 0:
                nc.vector.tensor_tensor(
                    acc[i][0:R, c0 : c0 + cw], pa[0:R, 0:cw], rb[:, 0:cw], mult
                )
            else:
                tmp = small.tile([R, 512], bf16, tag="tmp", name=f"tm{i}{j}{c0}")
                nc.vector.tensor_tensor(
                    tmp[:, 0:cw], pa[0:R, 0:cw], rb[:, 0:cw], mult
                )
                nc.gpsimd.tensor_tensor(
                    acc[i][0:R, c0 : c0 + cw],
                    acc[i][0:R, c0 : c0 + cw],
                    tmp[:, 0:cw],
                    add,
                )
            if last:
                emit_final_chunk(c0, cw)

        def emit_final_chunk(c0, cw):
            # the mm1 ring is idle in the tail; one slot holds both c-tiles
            fsl = ring.tile([P, 2, 512], f32, tag="ring", name=f"po{c0}")
            for mtile in range(KT):
                po = fsl[:, mtile, :]
                for i in range(3):
                    nc.tensor.matmul(
                        po[:, 0:cw],
                        woT_sb[i][:, mtile * P : (mtile + 1) * P],
                        acc[i][:, c0 : c0 + cw],
                        start=(i == 0),
                        stop=(i == 2),
                    )
                sig = small.tile([P, 512], f32, tag="sig", name=f"sg{mtile}{c0}")
                nc.scalar.activation(sig[:, 0:cw], po[:, 0:cw], Sigmoid)
                y_sb = small.tile([P, 512], f32, tag="ysb", name=f"yb{mtile}{c0}")
                nc.gpsimd.tensor_tensor(
                    y_sb[:, 0:cw],
                    xn_sb[:, mtile, c0 : c0 + cw],
                    sig[:, 0:cw],
                    mult,
                )
                nc.sync.dma_start(y_d[mtile][:, c0 : c0 + cw], y_sb[:, 0:cw])

        # ---- schedule ----
        pairs = [(i, j) for j in range(3) for i in range(3)]
        emit_q(0)
        emit_k_half(0, 0)
        emit_k_half(0, 1)
        for mt in range(6):
            emit_vT(mt)

        prev = None
        for idx, (i, j) in enumerate(pairs):
            E4c0 = e_pool.tile([P, MP, 1024], f8e4, tag="E4c0", name=f"E40_{idx}")
            E5c1 = e_pool.tile([P, MP, 1024], i8, tag="E5c1", name=f"E51_{idx}")
            E4c2 = e_pool.tile([P, MP, 256], f8e4, tag="E4c2", name=f"E42_{idx}")
            for mp in range(MP):
                emit_mm1_exp_mp(i, j, E4c0, E5c1, E4c2, mp)
                # interleave prev pair's mm2 so DVE norm ops land between exps
                if prev is not None:
                    emit_mm2_part(
                        prev[0], prev[1], prev[2], prev[3], prev[4], mp
                    )
                # off-critical projection work spread through early pairs
                if idx == 0:
                    if mp == 0:
                        emit_q(1)
                    elif mp == 4:
                        emit_q(2)
                    elif mp in (2, 3, 5, 6, 7, 8):
                        mt = 2 * mp + 2 if mp < 4 else 2 * mp
                        emit_vT(mt)
                        emit_vT(mt + 1)
                elif idx == 1 and mp == 2:
                    emit_k_half(1, 0)
                elif idx == 1 and mp == 6:
                    emit_k_half(1, 1)
                elif idx == 3 and mp == 2:
                    emit_k_half(2, 0)
                elif idx == 3 and mp == 6:
                    emit_k_half(2, 1)
            prev = (i, j, E4c0, E5c1, E4c2)
        for part in range(MP):
            emit_mm2_part(
                prev[0], prev[1], prev[2], prev[3], prev[4], part, last=True
            )

    nc.compile()
    return nc


def _get_program():
    if "nc" not in _CACHE:
        _CACHE["nc"] = _build_program()
    return _CACHE["nc"]


def _host_prep(x, wq, wk, wv, wo):
    import ml_dtypes

    bf16 = ml_dtypes.bfloat16
    f8 = ml_dtypes.float8_e4m3  # TRN FP8_EXP4 semantics (max 240)
    xf = np.ascontiguousarray(x.reshape(B, C, N), dtype=np.float32)

    # wq: [3, R, C] -> wqT: [C, 3, R] -> [KT, P, 3, R] zero-padded to 128 cols
    def wpad(w):
        wT = np.transpose(w, (2, 0, 1)).reshape(KT, P, 3, R)
        out = np.zeros((KT, P, 3, P), np.float32)
        out[:, :, :, 0:R] = wT
        return out.astype(f8)

    wqT = wpad(wq)
    wkT = wpad(wk)
    wvT = np.ascontiguousarray(
        np.transpose(wv, (2, 0, 1)).reshape(KT, P, 3, R)
    ).astype(f8)
    # wo: [C, 3R] -> woT[i] = wo[:, 64i:64(i+1)].T
    woT = np.ascontiguousarray(
        np.stack([wo[:, R * i : R * (i + 1)].T for i in range(3)])
    ).astype(bf16)
    in_maps = []
    for core in range(N_CORES):
        b, h = core // 2, core % 2
        xcore = xf[b].reshape(KT, P, N)
        xn32 = np.ascontiguousarray(xcore[:, :, h * NSLICE : (h + 1) * NSLICE])
        in_maps.append(
            {
                "xb": xcore.astype(f8),
                "xnb": xn32.astype(f8),
                "xn": xn32,
                "wqT": wqT,
                "wkT": wkT,
                "wvT": wvT,
                "woT": woT,
            }
        )
    return in_maps


def kernel(x, wq, wk, wv, wo):
    global LAST_RESULTS
    from concourse.bass_utils import run_bass_kernel_spmd

    x = np.asarray(x)
    nc = _get_program()
    in_maps = _host_prep(
        x, np.asarray(wq), np.asarray(wk), np.asarray(wv), np.asarray(wo)
    )
    res = run_bass_kernel_spmd(nc, in_maps, core_ids=list(range(N_CORES)))
    LAST_RESULTS = res
    out = np.empty((B, C, N), np.float32)
    for core in range(N_CORES):
        b, h = core // 2, core % 2
        out[b][:, h * NSLICE : (h + 1) * NSLICE] = res.results[core]["y"].reshape(
            C, NSLICE
        )
    return out.reshape(B, C, H, W).astype(x.dtype, copy=False)


# revision 42
# speedup vs baseline: 1.0296x; 1.0296x over previous
"""NestedAttention Trainium2 kernel (v3: row-tiled mm1, fp8 DoubleRow mm2,
exp split across Scalar+Vector engines per n-chunk).

Reference computation (per batch b):
  q_i = wq[i] @ x ; k_j = wk[j] @ x ; v_j = wv[j] @ x        (1x1 convs, r=64)
  for i: acc_i = sum_j softmax_m(q_i^T k_j / sqrt(r)) applied to v_j
  out = wo @ concat_i(acc_i) ; y = x * sigmoid(out)

Sharding: 8 cores = batch(4) x query-column-halves(2). Each core holds full
k/v (m = 2304 keys) and a 1152-wide slice of query columns n; no cross-core
communication (softmax is over m, fully on-core).

Per-core dataflow:
  mm1  T_ij[m, n] = k_j^T q_i  -- K=64 contraction, so the PE runs TWO
       concurrent 64x128 row-tiles (tile_position (0,0)/(64,0)); q/k are
       duplicated into partitions 64:128 (SBUF->SBUF DMA) to feed tile 1.
       Each (m-pair, chunk) lands in one 2-bank PSUM ring slot [128, 2, cw].
  exp  E = exp(0.125*T - 1.5), engine assigned PER N-CHUNK so ScalarE and
       VectorE drain ring slots concurrently:
         chunk0 (n 0:512)    -> ScalarE exact exp -> fp8e4
         chunk1 (n 512:1024) -> VectorE Schraudolph (int8 bitcast fp8e5),
                                except CH1_ACT m-pairs on ScalarE -> fp8e4
         chunk2 (n 1024:1152)-> ScalarE -> fp8e4
       The -1.5 bias cancels in softmax and keeps E < 240 (TRN e4m3 max).
  mm2  [vT_j | ones]^T @ E -- fp8 DoubleRow over m-slab pairs (FD=512
       chunks; the 128-wide chunk uses plain fp8 matmuls).  Rows 0:64 = v@E,
       rows 64:128 = Z (softmax denominator).
  norm acc_i += (v@E) * (1/Z): Z copy + reciprocal + multiply on VectorE,
       the j>0 accumulate add on GpSimd (Pool).
  out  wo^T @ acc (bf16), sigmoid (ScalarE), x*sig (VectorE), DMA out.
"""

import os
import numpy as np

B, C, H, W = 4, 256, 48, 48
N = H * W            # 2304 keys (m) per image
NSLICE = N // 2      # 1152 query columns (n) per core
R = 64               # reduced channels
P = 128
MT = N // P          # 18 m-tiles
MP = MT // 2         # 9 m-tile pairs (row-tiled mm1 does 2 at once)
KT = C // P          # 2 contraction tiles over channels
CHUNKS = [(0, 512), (512, 512), (1024, 128)]
N_CORES = 8

# chunk1 m-pairs whose exp runs on ScalarE instead of VectorE (load balance)
CH1_ACT = tuple(
    int(v) for v in os.environ.get("NESTED_CH1_ACT", "8").split(",") if v != ""
)
# chunk2 m-pairs whose exp runs on VectorE (default: none, all ScalarE)
CH2_DVE = tuple(
    int(v) for v in os.environ.get("NESTED_CH2_DVE", "").split(",") if v != ""
)

# exp(0.125*T + EBIAS); EBIAS cancels in softmax, gives fp8e4 overflow margin
EBIAS = -1.5
LOG2E = 1.4426950408889634
SCHRAUD_A = 4 * 0.125 * LOG2E
# 4*(15 + EBIAS*log2e) - 0.172 (sawtooth centering) + 0.25 (trunc/round split)
SCHRAUD_B = 4 * (15.0 + EBIAS * LOG2E) - 0.172 + 0.25

COPY1 = os.environ.get("NESTED_COPY1", "1") == "1"     # single 64-row Z copy
MIXED_DR = os.environ.get("NESTED_MIXED_DR", "0") == "1"  # e4 lhsT + e5 rhs

_CACHE = {}
LAST_RESULTS = None


def _build_program():
    from contextlib import ExitStack

    import concourse.bass as bass
    import concourse.tile as tile
    from concourse import bacc, mybir

    f32 = mybir.dt.float32
    bf16 = mybir.dt.bfloat16
    f8e4 = mybir.dt.float8e4
    f8e5 = mybir.dt.float8e5
    i8 = mybir.dt.int8
    Exp = mybir.ActivationFunctionType.Exp
    Sigmoid = mybir.ActivationFunctionType.Sigmoid
    Copy = mybir.ActivationFunctionType.Copy
    mult = mybir.AluOpType.mult
    add = mybir.AluOpType.add
    DR = mybir.MatmulPerfMode.DoubleRow

    nc = bacc.Bacc("TRN2", target_bir_lowering=False, debug=False)
    # fp8 inputs: the q/k/v projections contract K=256 = 2 slabs of 128, a
    # genuine fp8 DoubleRow case (one matmul per chunk instead of two)
    xb_d = nc.declare_dram_parameter("xb", [KT, P, N], f8e4, isOutput=False)
    xnb_d = nc.declare_dram_parameter("xnb", [KT, P, NSLICE], f8e4, isOutput=False)
    xn_d = nc.declare_dram_parameter("xn", [KT, P, NSLICE], f32, isOutput=False)
    # wq/wk transposed + zero-padded to 128 output cols so proj matmuls keep
    # the 128x128 PE mode (out partitions = 128, rows 64:128 zero)
    wqT_d = nc.declare_dram_parameter("wqT", [KT, P, 3, P], f8e4, isOutput=False)
    wkT_d = nc.declare_dram_parameter("wkT", [KT, P, 3, P], f8e4, isOutput=False)
    wvT_d = nc.declare_dram_parameter("wvT", [KT, P, 3, R], f8e4, isOutput=False)
    woT_d = nc.declare_dram_parameter("woT", [3, R, C], bf16, isOutput=False)
    y_d = nc.declare_dram_parameter("y", [KT, P, NSLICE], f32, isOutput=True)

    with tile.TileContext(nc) as tc, ExitStack() as ctx:
        consts = ctx.enter_context(tc.tile_pool(name="consts", bufs=1))
        ring = ctx.enter_context(tc.tile_pool(name="ring", bufs=3, space="PSUM"))
        mm2_ps = ctx.enter_context(tc.tile_pool(name="mm2_ps", bufs=2, space="PSUM"))
        e_pool = ctx.enter_context(tc.tile_pool(name="e_pool", bufs=3))
        small = ctx.enter_context(tc.tile_pool(name="small", bufs=2))

        # PE warm-up: dummy matmuls ramp the HAM clock gate to 2.4 GHz while
        # the framework preamble and input DMAs are still running.
        warm = consts.tile([P, 512], bf16, name="warm")
        nc.gpsimd.memset(warm[:], 0.0)
        for w in range(12):
            wsl = ring.tile([P, 2, 512], f32, tag="ring", name=f"wu{w}")
            nc.tensor.matmul(
                wsl[:, 0, :], warm[:, 0:128], warm[:], start=True, stop=True
            )

        # ---- persistent SBUF state ----
        wqT_sb = consts.tile([P, KT, 3, P], f8e4)
        nc.sync.dma_start(wqT_sb[:], wqT_d.rearrange("t p i r -> p t i r"))
        xnb_sb = consts.tile([P, KT, NSLICE], f8e4)
        nc.sync.dma_start(xnb_sb[:], xnb_d.rearrange("t p m -> p t m"))
        wkT_sb = consts.tile([P, KT, 3, P], f8e4)
        nc.sync.dma_start(wkT_sb[:], wkT_d.rearrange("t p i r -> p t i r"))
        x_sb = consts.tile([P, KT, N], f8e4)
        for q4 in range(2):
            n0, n1 = q4 * NSLICE, (q4 + 1) * NSLICE
            nc.sync.dma_start(
                x_sb[:, :, n0:n1], xb_d[:, :, n0:n1].rearrange("t p m -> p t m")
            )
        wvT_sb = consts.tile([P, KT, 3, R], f8e4)
        nc.sync.dma_start(wvT_sb[:], wvT_d.rearrange("t p i r -> p t i r"))
        xn_sb = consts.tile([P, KT, NSLICE], f32)
        for q4 in range(2):
            n0, n1 = q4 * 576, (q4 + 1) * 576
            nc.sync.dma_start(
                xn_sb[:, :, n0:n1], xn_d[:, :, n0:n1].rearrange("t p m -> p t m")
            )

        woT_sb = []
        for i in range(3):
            w = consts.tile([P, C], bf16, tag=f"woT{i}", name=f"woT{i}")
            nc.gpsimd.memset(w[R:P, :], 0.0)
            nc.sync.dma_start(w[0:R, :], woT_d[i])
            woT_sb.append(w)

        # q/k: rows 0:64 = values, rows 64:128 = duplicate (feeds row-tile 1)
        q_sb = consts.tile([P, 3, NSLICE], bf16)
        k_sb = consts.tile([P, 3, N], bf16)

        # vT buffers per m-pair: [2 slabs, [vT_0|ones|vT_1|ones|vT_2|ones]]
        # e4m3 copy feeds chunk0/2 matmuls, e5m2 copy feeds chunk1.
        vT4 = consts.tile([P, MP, 2, 384], f8e4)
        for j in range(3):
            nc.gpsimd.memset(vT4[:, :, :, 128 * j + 64 : 128 * j + 128], 1.0)
        if not MIXED_DR:
            vT5 = consts.tile([P, MP, 2, 384], f8e5)
            for j in range(3):
                nc.gpsimd.memset(vT5[:, :, :, 128 * j + 64 : 128 * j + 128], 1.0)

        ebias_ap = consts.tile([P, 1], f32, name="ebias")
        nc.gpsimd.memset(ebias_ap[:], EBIAS)

        # acc_i accumulated in bf16; rows 64:128 zero (K=128 pad for final mm)
        acc = []
        for i in range(3):
            a = consts.tile([P, NSLICE], bf16, tag=f"acc{i}", name=f"acc{i}")
            nc.gpsimd.memset(a[R:P, :], 0.0)
            acc.append(a)

        # ---- projections (use the mm2 PSUM pool; never touch the mm1 ring) ----
        def _proj_1152(wT_sb, wi, src_sb, src_base, dst, name):
            """dst[0:64, 0:1152] (strided AP ok) = w.T @ src columns.
            fp8 DoubleRow over the two K=128 channel slabs."""
            for ci, (c0, cw) in enumerate(CHUNKS):
                pt = mm2_ps.tile([P, 512], f32, tag="mm2", name=f"{name}{ci}")
                nc.tensor.matmul(
                    pt[:, 0:cw],
                    wT_sb[:, :, wi, :],
                    src_sb[:, :, src_base + c0 : src_base + c0 + cw],
                    start=True,
                    stop=True,
                    perf_mode=DR,
                )
                nc.vector.tensor_copy(dst[0:R, c0 : c0 + cw], pt[0:R, 0:cw])

        def emit_q(i):
            _proj_1152(wqT_sb, i, xnb_sb, 0, q_sb[:, i, :], f"qp{i}")
            nc.sync.dma_start(q_sb[R:P, i, :], q_sb[0:R, i, :])

        def emit_k_half(j, half):
            n0 = half * NSLICE
            _proj_1152(wkT_sb, j, x_sb, n0, k_sb[:, j, n0 : n0 + NSLICE], f"kp{j}{half}")
            nc.sync.dma_start(
                k_sb[R:P, j, n0 : n0 + NSLICE], k_sb[0:R, j, n0 : n0 + NSLICE]
            )

        def emit_vT(mt):
            mp, s = mt // 2, mt % 2
            pv = mm2_ps.tile([P, 512], f32, tag="mm2", name=f"pv{mt}")
            nc.tensor.matmul(
                pv[:, 0 : 3 * R],
                x_sb[:, :, mt * P : (mt + 1) * P],
                wvT_sb[:, :, :, :],
                start=True,
                stop=True,
                perf_mode=DR,
            )
            src = pv[:, 0 : 3 * R].rearrange("p (j r) -> p j r", j=3)
            targets = [(vT4, True)] if MIXED_DR else [(vT4, True), (vT5, False)]
            for buf, on_act in targets:
                base = buf[:, mp, s, :]
                dst = bass.AP(
                    tensor=base.tensor,
                    offset=base.offset,
                    ap=[base.ap[0], [128, 3], [1, R]],
                )
                if on_act:
                    nc.scalar.activation(dst, src, Copy)
                else:
                    nc.vector.tensor_copy(dst, src)

        # ---- attention ----
        c2_state = {}

        def emit_mm1_exp_mp(i, j, E4c0, E5c1, E4c2, mp):
            """Row-tiled mm1 (2 concurrent 64x128 tiles) + per-slot exp for
            one m-pair.

            E layouts keep each exp instruction's input AND output fully
            contiguous: E4c0/E5c1 are [P, mp, 1024] = [tileA 512 | tileB 512],
            E4c2 is [P, mp, 256] = [tileA 128 | tileB 128].  The 128-wide
            chunk2 outputs of four m-pairs park in ONE ring slot and share a
            single exp instruction (amortizes ScalarE access latency).
            """
            mA, mB = 2 * mp, 2 * mp + 1
            for ci, (c0, cw) in enumerate(CHUNKS[:2]):
                sl = ring.tile(
                    [P, 2, 512], f32, tag="ring", name=f"t{i}{j}m{mp}c{c0}"
                )
                nc.tensor.matmul(
                    sl[:, 0, 0:cw],
                    k_sb[0:R, j, mA * P : (mA + 1) * P],
                    q_sb[0:R, i, c0 : c0 + cw],
                    start=True,
                    stop=True,
                    tile_position=(0, 0),
                )
                nc.tensor.matmul(
                    sl[:, 1, 0:cw],
                    k_sb[R:P, j, mB * P : (mB + 1) * P],
                    q_sb[R:P, i, c0 : c0 + cw],
                    start=True,
                    stop=True,
                    tile_position=(64, 0),
                )
                on_dve = (ci == 1) and (mp not in CH1_ACT)
                src = sl[:, :, :].rearrange("p s c -> p (s c)")
                if ci == 0:
                    dst4 = E4c0[:, mp, :]
                    dst5 = dst4.bitcast(i8)
                else:
                    dst5 = E5c1[:, mp, :]
                    dst4 = dst5.bitcast(f8e4)
                if on_dve:
                    nc.vector.tensor_scalar(
                        dst5, src, SCHRAUD_A, SCHRAUD_B, mult, add
                    )
                else:
                    nc.scalar.activation(
                        dst4, src, Exp, bias=ebias_ap[:], scale=0.125
                    )
            # chunk2 (n 1024:1152); the spare slot region hosts HAM
            # warm-keeper dummies so PE micro-idles at m-pair boundaries
            # don't re-throttle the clock gate
            c0, cw = CHUNKS[2]
            sl = ring.tile([P, 2, 512], f32, tag="ring", name=f"t{i}{j}m{mp}c2")
            nc.tensor.matmul(
                sl[:, 0, 0:cw],
                k_sb[0:R, j, mA * P : (mA + 1) * P],
                q_sb[0:R, i, c0 : c0 + cw],
                start=True,
                stop=True,
                tile_position=(0, 0),
            )
            nc.tensor.matmul(
                sl[:, 1, 0:cw],
                k_sb[R:P, j, mB * P : (mB + 1) * P],
                q_sb[R:P, i, c0 : c0 + cw],
                start=True,
                stop=True,
                tile_position=(64, 0),
            )
            for s in range(2):
                nc.tensor.matmul(
                    sl[:, s, 384:512],
                    warm[:, 0:128],
                    warm[:, 0:128],
                    start=True,
                    stop=True,
                )
            dst4 = E4c2[:, mp, :].rearrange("p (s c) -> p s c", s=2)
            nc.scalar.activation(
                dst4, sl[:, :, 0:cw], Exp, bias=ebias_ap[:], scale=0.125
            )

        mm2_state = {}

        def emit_mm2_part(i, j, E4c0, E5c1, E4c2, part, last=False):
            """One third of a chunk's mm2 accumulation chain (interleaved
            between m-pairs so the PE never bursts long enough to drain the
            mm1 ring); the norm runs on the closing part."""
            ci, step = part // 3, part % 3
            c0, cw = CHUNKS[ci]
            if step == 0:
                mm2_state[ci] = mm2_ps.tile(
                    [P, 512], f32, tag="mm2", name=f"pa{i}{j}{c0}"
                )
            pa = mm2_state[ci]
            if ci < 2:
                Ec = E4c0 if ci == 0 else E5c1
                for mp in range(3 * step, 3 * step + 3):
                    dve = (ci == 1) and (mp not in CH1_ACT)
                    slabs = Ec[:, mp, :].rearrange("p (s c) -> p s c", s=2)
                    if dve:
                        lhsT = (vT4 if MIXED_DR else vT5)[
                            :, mp, :, 128 * j : 128 * (j + 1)
                        ]
                        rhs = slabs.bitcast(f8e5)
                    else:
                        lhsT = vT4[:, mp, :, 128 * j : 128 * (j + 1)]
                        rhs = slabs.bitcast(f8e4) if ci == 1 else slabs
                    nc.tensor.matmul(
                        pa[:, 0:cw],
                        lhsT,
                        rhs,
                        start=(mp == 0),
                        stop=(mp == MP - 1),
                        perf_mode=DR,
                    )
            else:
                # FD=128: DoubleRow loses to FWL; plain fp8 per slab
                for mt in range(6 * step, 6 * step + 6):
                    mp, s = mt // 2, mt % 2
                    if mp in CH2_DVE:
                        lhsT = (vT4 if MIXED_DR else vT5)[
                            :, mp, s, 128 * j : 128 * (j + 1)
                        ]
                        rhs = E4c2[:, mp, 128 * s : 128 * (s + 1)].bitcast(f8e5)
                    else:
                        lhsT = vT4[:, mp, s, 128 * j : 128 * (j + 1)]
                        rhs = E4c2[:, mp, 128 * s : 128 * (s + 1)]
                    nc.tensor.matmul(
                        pa[:, 0:cw],
                        lhsT,
                        rhs,
                        start=(mt == 0),
                        stop=(mt == MT - 1),
                    )
            if step < 2:
                return
            rb = small.tile([R, 512], f32, tag="rb", name=f"rb{i}{j}{c0}")
            if COPY1:
                nc.vector.tensor_copy(rb[0:R, 0:cw], pa[R:P, 0:cw])
            else:
                nc.vector.tensor_copy(rb[0:32, 0:cw], pa[64:96, 0:cw])
                nc.vector.tensor_copy(rb[32:64, 0:cw], pa[96:128, 0:cw])
            nc.vector.reciprocal_approx_fast(rb[:, 0:cw], rb[:, 0:cw])
            if j == 0:
                nc.vector.tensor_tensor(
                    acc[i][0:R, c0 : c0 + cw], pa[0:R, 0:cw], rb[:, 0:cw], mult
                )
            else:
                tmp = small.tile([R, 512], bf16, tag="tmp", name=f"tm{i}{j}{c0}")
                nc.vector.tensor_tensor(
                    tmp[:, 0:cw], pa[0:R, 0:cw], rb[:, 0:cw], mult
                )
                nc.gpsimd.tensor_tensor(
                    acc[i][0:R, c0 : c0 + cw],
                    acc[i][0:R, c0 : c0 + cw],
                    tmp[:, 0:cw],
                    add,
                )
            if last:
                emit_final_chunk(c0, cw)

        def emit_final_chunk(c0, cw):
            # the mm1 ring is idle in the tail; one slot holds both c-tiles
            fsl = ring.tile([P, 2, 512], f32, tag="ring", name=f"po{c0}")
            for mtile in range(KT):
                po = fsl[:, mtile, :]
                for i in range(3):
                    nc.tensor.matmul(
                        po[:, 0:cw],
                        woT_sb[i][:, mtile * P : (mtile + 1) * P],
                        acc[i][:, c0 : c0 + cw],
                        start=(i == 0),
                        stop=(i == 2),
                    )
                sig = small.tile([P, 512], f32, tag="sig", name=f"sg{mtile}{c0}")
                nc.scalar.activation(sig[:, 0:cw], po[:, 0:cw], Sigmoid)
                y_sb = small.tile([P, 512], f32, tag="ysb", name=f"yb{mtile}{c0}")
                nc.gpsimd.tensor_tensor(
                    y_sb[:, 0:cw],
                    xn_sb[:, mtile, c0 : c0 + cw],
                    sig[:, 0:cw],
                    mult,
                )
                nc.sync.dma_start(y_d[mtile][:, c0 : c0 + cw], y_sb[:, 0:cw])

        # ---- schedule ----
        pairs = [(i, j) for j in range(3) for i in range(3)]
        emit_q(0)
        emit_k_half(0, 0)
        emit_k_half(0, 1)
        for mt in range(6):
            emit_vT(mt)

        prev = None
        for idx, (i, j) in enumerate(pairs):
            E4c0 = e_pool.tile([P, MP, 1024], f8e4, tag="E4c0", name=f"E40_{idx}")
            E5c1 = e_pool.tile([P, MP, 1024], i8, tag="E5c1", name=f"E51_{idx}")
            E4c2 = e_pool.tile([P, MP, 256], f8e4, tag="E4c2", name=f"E42_{idx}")
            for mp in range(MP):
                emit_mm1_exp_mp(i, j, E4c0, E5c1, E4c2, mp)
                # interleave prev pair's mm2 so DVE norm ops land between exps
                if prev is not None:
                    emit_mm2_part(
                        prev[0], prev[1], prev[2], prev[3], prev[4], mp
                    )
                # off-critical projection work spread through early pairs
                if idx == 0:
                    if mp == 0:
                        emit_q(1)
                    elif mp == 4:
                        emit_q(2)
                    elif mp in (2, 3, 5, 6, 7, 8):
                        mt = 2 * mp + 2 if mp < 4 else 2 * mp
                        emit_vT(mt)
                        emit_vT(mt + 1)
                elif idx == 1 and mp == 2:
                    emit_k_half(1, 0)
                elif idx == 1 and mp == 6:
                    emit_k_half(1, 1)
                elif idx == 3 and mp == 2:
                    emit_k_half(2, 0)
                elif idx == 3 and mp == 6:
                    emit_k_half(2, 1)
            prev = (i, j, E4c0, E5c1, E4c2)
        for part in range(MP):
            emit_mm2_part(
                prev[0], prev[1], prev[2], prev[3], prev[4], part, last=True
            )

    nc.compile()
    return nc


def _get_program():
    if "nc" not in _CACHE:
        _CACHE["nc"] = _build_program()
    return _CACHE["nc"]


def _host_prep(x, wq, wk, wv, wo):
    import ml_dtypes

    bf16 = ml_dtypes.bfloat16
    f8 = ml_dtypes.float8_e4m3  # TRN FP8_EXP4 semantics (max 240)
    xf = np.ascontiguousarray(x.reshape(B, C, N), dtype=np.float32)

    # wq: [3, R, C] -> wqT: [C, 3, R] -> [KT, P, 3, R] zero-padded to 128 cols
    def wpad(w):
        wT = np.transpose(w, (2, 0, 1)).reshape(KT, P, 3, R)
        out = np.zeros((KT, P, 3, P), np.float32)
        out[:, :, :, 0:R] = wT
        return out.astype(f8)

    wqT = wpad(wq)
    wkT = wpad(wk)
    wvT = np.ascontiguousarray(
        np.transpose(wv, (2, 0, 1)).reshape(KT, P, 3, R)
    ).astype(f8)
    # wo: [C, 3R] -> woT[i] = wo[:, 64i:64(i+1)].T
    woT = np.ascontiguousarray(
        np.stack([wo[:, R * i : R * (i + 1)].T for i in range(3)])
    ).astype(bf16)
    in_maps = []
    for core in range(N_CORES):
        b, h = core // 2, core % 2
        xcore = xf[b].reshape(KT, P, N)
        xn32 = np.ascontiguousarray(xcore[:, :, h * NSLICE : (h + 1) * NSLICE])
        in_maps.append(
            {
                "xb": xcore.astype(f8),
                "xnb": xn32.astype(f8),
                "xn": xn32,
                "wqT": wqT,
                "wkT": wkT,
                "wvT": wvT,
                "woT": woT,
            }
        )
    return in_maps


def kernel(x, wq, wk, wv, wo):
    global LAST_RESULTS
    from concourse.bass_utils import run_bass_kernel_spmd

    x = np.asarray(x)
    nc = _get_program()
    in_maps = _host_prep(
        x, np.asarray(wq), np.asarray(wk), np.asarray(wv), np.asarray(wo)
    )
    res = run_bass_kernel_spmd(nc, in_maps, core_ids=list(range(N_CORES)))
    LAST_RESULTS = res
    out = np.empty((B, C, N), np.float32)
    for core in range(N_CORES):
        b, h = core // 2, core % 2
        out[b][:, h * NSLICE : (h + 1) * NSLICE] = res.results[core]["y"].reshape(
            C, NSLICE
        )
    return out.reshape(B, C, H, W).astype(x.dtype, copy=False)
